# revision 21
# baseline (speedup 1.0000x reference)
"""Batch contrastive loss (InfoNCE over a 4096x4096 score matrix) on 8 trn2 cores.

scores = nl_vec @ code_vec.T  [4096, 4096]
loss   = -mean(log_softmax(scores)[i, i])
       = mean_i( logsumexp_j scores[i, j] - scores[i, i] )

Sharding: each core owns a 512-row block of nl_vec and computes its block of
scores against the full code_vec (tensor-parallel GEMM), then local CE row
stats; the per-core stats are merged on host (all-reduce mean).

Device layout choices:
- Both GEMM operands enter the PE with the contraction dim (d=768) on
  partitions, so the host supplies pre-transposed views (codeT = code.T,
  nlT = nl_slice.T). GEMM inputs are bf16 (input rounding moves this loss by
  ~7e-6 relative; PSUM accumulation and all score-space math stay fp32).
- codeT is rotated per-core by -512*c columns so each core's "own" diagonal
  block lands at columns [0, 512) of its score block. Softmax row stats are
  permutation-invariant, and the diag extraction offset becomes per-core
  constant, keeping the program SPMD-identical across cores.
- Matmuls are ordered k-major inside each 1024-column block so the PE can
  start as soon as the first contraction slice of a block lands, instead of
  stalling on the whole block's DMA.
- Per 1024-col PSUM group: DVE computes the (negated) column-block max, ACT
  computes exp(x - max) with a fused row-sum (accum_out). The per-block
  (max, sumexp) pairs and the diagonal go back to the host, which does the
  standard logsumexp block merge: per-core work there is a [128, 16] merge —
  microseconds of numpy — and it keeps the Exp->Ln activation-table switch
  (~2.7us) and a serial reduction tail off the device's critical path.
"""

import sys

if "/opt/trn_rl_repo" not in sys.path:
    sys.path.insert(0, "/opt/trn_rl_repo")

import numpy as np

BS = 4096
D = 768
NCORES = 8
R = BS // NCORES  # 512 rows per core
P = 128
KT = D // P       # 6 contraction tiles
NT = R // P       # 4 row-tiles per core
JW = 1024         # column-block width (= one PSUM group, 2 banks)
NJB = BS // JW    # 4 column blocks

_CACHE = {}


def build_nc():
    if "nc" in _CACHE:
        return _CACHE["nc"]

    from contextlib import ExitStack

    import concourse.bacc as bacc
    import concourse.mybir as mybir
    import concourse.tile as tile

    f32 = mybir.dt.float32
    bf16 = mybir.dt.bfloat16
    AF = mybir.ActivationFunctionType
    ALU = mybir.AluOpType
    AX = mybir.AxisListType

    nc = bacc.Bacc(
        "TRN2", debug=False, target_bir_lowering=False, num_devices=NCORES
    )
    codeT_d = nc.dram_tensor("codeT", [D, BS], bf16, kind="ExternalInput").ap()
    nlT_d = nc.dram_tensor("nlT", [D, R], bf16, kind="ExternalInput").ap()
    ident_d = nc.dram_tensor("ident", [P, P], f32, kind="ExternalInput").ap()
    # stats out, one tensor: NB 1024-wide score blocks per row-tile.
    # cols [0,16) negated per-block max, [16,32) per-block sumexp (column
    # index inside each half = t*NB + jb), [32,36) diag per row-tile.
    NB = NJB
    stat_d = nc.dram_tensor(
        "statout", [P, 2 * NT * NB + NT], f32, kind="ExternalOutput"
    ).ap()

    with ExitStack() as ctx:
        tc = ctx.enter_context(tile.TileContext(nc))
        code_pool = ctx.enter_context(tc.tile_pool(name="code", bufs=1))
        nl_pool = ctx.enter_context(tc.tile_pool(name="nl", bufs=1))
        const_pool = ctx.enter_context(tc.tile_pool(name="const", bufs=1))
        ps_pool = ctx.enter_context(tc.tile_pool(name="ps", bufs=4, space="PSUM"))
        scr_pool = ctx.enter_context(tc.tile_pool(name="scr", bufs=4))
        stat_pool = ctx.enter_context(tc.tile_pool(name="stat", bufs=1))

        # DMA issue order is arrival order: pair each contraction slice of
        # the first column block with its nlT slice so the first PSUM group
        # can close as early as possible; remaining blocks follow jb-major.
        # The identity (64KB) goes first so the jb-0 diag never blocks the
        # in-order DVE stream.
        nt_sb = [
            nl_pool.tile([P, R], bf16, tag=f"nlT_{k}", name=f"nlT_sb_{k}")
            for k in range(KT)
        ]
        ct_sb = {
            (k, jb): code_pool.tile(
                [P, JW], bf16, tag=f"ct_{k}_{jb}", name=f"ct_sb_{k}_{jb}"
            )
            for jb in range(NJB)
            for k in range(KT)
        }
        ident = const_pool.tile([P, P], f32, tag="ident", name="ident_sb")
        for k in range(KT):
            nc.sync.dma_start(nt_sb[k][:], nlT_d[k * P : (k + 1) * P, :])
            nc.sync.dma_start(ct_sb[(k, 0)][:], codeT_d[k * P : (k + 1) * P, 0:JW])
            if k == 0:
                # ident is only needed once the first group closes; slot it
                # behind the first matmul's operands.
                nc.sync.dma_start(ident[:], ident_d[:, :])
        for jb in range(1, NJB):
            for k in range(KT):
                nc.sync.dma_start(
                    ct_sb[(k, jb)][:],
                    codeT_d[k * P : (k + 1) * P, jb * JW : (jb + 1) * JW],
                )
        STAT = stat_pool.tile(
            [P, 2 * NT * NB + NT], f32, tag="stat", name="stat_sb"
        )
        M32 = STAT[:, 0 : NT * NB]
        S32 = STAT[:, NT * NB : 2 * NT * NB]
        DG4 = STAT[:, 2 * NT * NB : 2 * NT * NB + NT]

        for jb in range(NJB):
            pss = [
                ps_pool.tile([P, JW], f32, tag="ps", name=f"ps_{jb}_{t}")
                for t in range(NT)
            ]
            # t-major: one row-tile's full contraction at a time, so groups
            # complete staggered and PSUM banks recycle smoothly.
            order = [(k, t) for t in range(NT) for k in range(KT)]
            for k, t in order:
                for h in range(JW // 512):
                    nc.tensor.matmul(
                        pss[t][:, h * 512 : (h + 1) * 512],
                        nt_sb[k][:, t * P : (t + 1) * P],
                        ct_sb[(k, jb)][:, h * 512 : (h + 1) * 512],
                        start=(k == 0),
                        stop=(k == KT - 1),
                    )
            for t in range(NT):
                ps = pss[t]
                if jb == 0:
                    # own-block diagonal: element (p, t*128+p). Plain DVE
                    # mul+reduce — tensor_tensor_reduce with a PSUM operand
                    # faults the exec unit (NRT status 101) on this toolchain.
                    scr128 = scr_pool.tile(
                        [P, P], f32, tag="scr128", name=f"scr128_{t}"
                    )
                    nc.vector.tensor_mul(
                        scr128[:], ps[:, t * P : (t + 1) * P], ident[:]
                    )
                    nc.vector.tensor_reduce(
                        out=DG4[:, t : t + 1],
                        in_=scr128[:],
                        axis=AX.X,
                        op=ALU.add,
                    )
                col = t * NB + jb
                nc.vector.tensor_reduce(
                    out=M32[:, col : col + 1],
                    in_=ps[:],
                    axis=AX.X,
                    op=ALU.max,
                    negate=True,
                )
                scr = scr_pool.tile(
                    [P, JW], f32, tag="scr1024", name=f"scr1024_{jb}_{t}"
                )
                nc.scalar.activation(
                    scr[:],
                    ps[:],
                    AF.Exp,
                    bias=M32[:, col : col + 1],
                    scale=1.0,
                    accum_out=S32[:, col : col + 1],
                )

        nc.sync.dma_start(stat_d[:, :], STAT[:])

    nc.compile()
    _CACHE["nc"] = nc
    return nc


def make_in_maps(code_vec: np.ndarray, nl_vec: np.ndarray):
    import ml_dtypes

    bf = ml_dtypes.bfloat16
    code_vec = np.ascontiguousarray(np.asarray(code_vec, dtype=np.float32))
    nl_vec = np.ascontiguousarray(np.asarray(nl_vec, dtype=np.float32))
    assert code_vec.shape == (BS, D) and nl_vec.shape == (BS, D)
    codeT = code_vec.T.astype(bf)  # [D, BS]
    ident = np.eye(P, dtype=np.float32)
    in_maps = []
    for c in range(NCORES):
        codeT_rot = np.ascontiguousarray(np.roll(codeT, -c * R, axis=1))
        nlT = np.ascontiguousarray(nl_vec[c * R : (c + 1) * R, :].T.astype(bf))
        in_maps.append({"codeT": codeT_rot, "nlT": nlT, "ident": ident})
    return in_maps


def merge_stats(results):
    """Host-side logsumexp block merge of the per-core stats -> loss sum."""
    total = 0.0
    NB = BS // JW
    nb = NT * NB
    for r in results:
        st = r["statout"].astype(np.float64)
        negm = st[:, 0:nb].reshape(P, NT, NB)
        s = st[:, nb : 2 * nb].reshape(P, NT, NB)
        dg = st[:, 2 * nb : 2 * nb + NT]  # [P, NT]
        m = -negm  # per-block max, [P, NT, NJB]
        mstar = m.max(axis=2)  # [P, NT]
        sstar = (s * np.exp(m - mstar[:, :, None])).sum(axis=2)
        lse = mstar + np.log(sstar)
        total += (lse - dg).sum()
    return total


def kernel(code_vec, nl_vec, bs=None, **_ignored):
    from concourse import bass_utils

    nc = build_nc()
    in_maps = make_in_maps(code_vec, nl_vec)
    res = bass_utils.run_bass_kernel_spmd(
        nc, in_maps, core_ids=list(range(NCORES))
    )
    loss = np.float32(merge_stats(res.results) / BS)
    return np.asarray(loss, dtype=np.float32)
